# revision 4
# baseline (speedup 1.0000x reference)
"""Trainium2 Bass kernel for ColorImageLoss (gaussian-blur + bilinear grid
sample + MSE), data-parallel over batch across 8 NeuronCores.

The loss reads the blurred image only at 64 sample points per image; each
sample needs an 8x8 (blur support) x 3ch patch of the raw image, with the
7-tap blur + bilinear weights folded into per-sample 8-tap row/col weight
vectors (reflected tap indices provably stay inside the clamped 8-wide
window [clamp(x0-3,0,W-8), +8)).

Layout trick: the host re-lays the image into overlapping 8-row bands
  D[band y0][x][row r][ch]  (band stride 1 row, 505 bands)
so the whole 8x8x3 patch of a sample is ONE contiguous 192-element run at
element offset  img_base + y0*12288 + x0*24.  The gather is then 2 indirect
DMAs (one per 128-sample slot, one descriptor per partition) instead of 48 —
SWDGE descriptor-generation fixed cost (~1us/call) dominated the baseline.

Weight trick: instead of scattering 7-tap kernels for x0 and x1 separately
and blending, use the blended 8-tap kernel  kkb[u] = kk8[u] + w*(kk8[u-1]
- kk8[u])  on raw taps T[u] = floor-3+u (exact also at the border clamp,
where w=0 kills the phantom tap), reflect T, and scatter once per axis.

All gather indices are exact in f32: offsets are multiples of 24 below 2^25.
"""

import os
import sys

import numpy as np

for _p in ("/opt/trn_rl_repo", "/root/.axon_site/_ro/trn_rl_repo"):
    if os.path.isdir(_p) and _p not in sys.path:
        sys.path.insert(0, _p)

import concourse.bass as bass
import concourse.mybir as mybir
import concourse.tile as tile
from concourse.bass_utils import run_bass_kernel_spmd

# Problem geometry (hardcoded per contract)
B, L, NCH, H, W = 32, 64, 3, 512, 512
NCORES = 8
BPC = B // NCORES            # images per core
NS = BPC * L                 # samples per core (256)
P = 128                      # SBUF partitions
SLOTS = NS // P              # 2 sample slots per partition
KS = 7                       # blur taps
RB = 8                       # rows per band
NB = H - KS                  # 505 bands (window start sy in [0, 504])
XS = RB * NCH                # 24  elements per x column of a band
BSTR = W * XS                # 12288 elements per band
IMS = NB * BSTR              # 6_205_440 elements per image
TOT = BPC * IMS              # banded-image elements per core
PATCH = 8 * RB * NCH         # 192 contiguous elements per sample

f32 = mybir.dt.float32
i32 = mybir.dt.int32
Alu = mybir.AluOpType
Ax = mybir.AxisListType

# meta tensor per-partition layout (f32 columns)
O_POS = 0            # [SLOTS, 2] (x, y)                  -> 4
O_COL = 4            # [SLOTS, 3] color                   -> 6
O_KK8 = 10           # [8] blur kernel, kk8[7] = 0        -> 8
O_DK8 = 18           # [8] kk8[u-1] - kk8[u]              -> 8
O_IM3 = 26           # [8] u - 3                          -> 8
O_IOTA = 34          # [8] 0..7                           -> 8
O_IB = 42            # [SLOTS] image base element offset  -> 2
META_W = 44


def _gauss_kernel_np():
    x = (np.arange(KS, dtype=np.float32) - (KS - 1) / 2).astype(np.float32)
    k = np.exp(-0.5 * (x / np.float32(1.0)) ** 2).astype(np.float32)
    return (k / k.sum()).astype(np.float32)


def _fap(t, dims, extra_offset=0):
    """AP over tile `t` keeping its partition dim, replacing free dims.

    dims: list of [step, count] in elements; step 0 broadcasts.
    """
    base = t[:] if hasattr(t, "tile") else t
    return bass.AP(
        base.tensor, base.offset + extra_offset,
        [list(base.ap[0])] + [list(d) for d in dims],
    )


def split_multi_waits(nc):
    """This walrus encodes at most ONE sync wait per TPB instruction.  Hoist
    extra waits onto same-engine NoOps inserted directly before the
    instruction (the sequencer executes waits in queue order, so semantics
    are identical)."""
    n_split = 0
    for f in nc.m.functions:
        for blk in f.blocks:
            insts = blk.instructions
            i = 0
            while i < len(insts):
                inst = insts[i]
                si = inst.sync_info
                if si is not None and si.on_wait is not None and len(si.on_wait) > 1:
                    waits = list(si.on_wait)
                    for w in waits[:-1]:
                        nop = mybir.InstNoOp(
                            name=f"{inst.name}-wsplit{n_split}",
                            engine=inst.engine,
                            ins=[],
                            outs=[],
                            sync_info=mybir.SyncInfo(on_wait=[w], on_update=[]),
                        )
                        nc.register_instruction(nop, overwrite=True)
                        insts.insert(i, nop)
                        i += 1
                        n_split += 1
                    inst.sync_info = mybir.SyncInfo(
                        on_wait=[waits[-1]], on_update=list(si.on_update or []))
                i += 1
    return n_split


def build_bass(repeat=1):
    nc = bass.Bass("TRN2")

    img = nc.dram_tensor("img", [TOT, 1], f32, kind="ExternalInput")
    meta = nc.dram_tensor("meta", [P, META_W], f32, kind="ExternalInput")
    partial = nc.dram_tensor("partial", [P, 1], f32, kind="ExternalOutput")

    v = None  # set per rep
    with tile.TileContext(nc) as tc:
        for _rep in range(repeat):
            with tc.tile_pool(name=f"main{_rep}", bufs=1) as pool:
                m = pool.tile([P, META_W], f32)
                nc.sync.dma_start(out=m[:], in_=meta[:])
                v = nc.vector     # DVE: front chain + weight chain
                g = nc.gpsimd     # Pool: gathers + post-gather tail

                # ---- positions -> pixel coords: clamp(512*p - 0.5, 0, 511)
                # (fused form of the reference op order; deviates by <=1ulp
                # which only matters within ~6e-5 px of integer boundaries
                # where the bilinear interpolant is continuous anyway) ----
                xy = pool.tile([P, 2 * SLOTS], f32)       # [slot][axis(x,y)]
                nc_pos = _fap(m, [[1, 2 * SLOTS]], O_POS)
                v.tensor_scalar(xy[:], nc_pos, float(W), -0.5, Alu.mult, Alu.add)
                v.tensor_scalar(xy[:], xy[:], 0.0, float(W - 1), Alu.max, Alu.min)

                # ---- floor/frac via round-to-nearest trick ----
                rnd = pool.tile([P, 2 * SLOTS], f32)
                v.tensor_scalar(rnd[:], xy[:], 8388608.0, None, Alu.add)
                v.tensor_scalar(rnd[:], rnd[:], -8388608.0, None, Alu.add)
                gtx = pool.tile([P, 2 * SLOTS], f32)
                v.tensor_tensor(gtx[:], rnd[:], xy[:], op=Alu.is_gt)
                fxy = pool.tile([P, 2 * SLOTS], f32)      # floor  [slot][ax]
                wxy = pool.tile([P, 2 * SLOTS], f32)      # frac   [slot][ax]
                v.tensor_sub(fxy[:], rnd[:], gtx[:])
                v.tensor_sub(wxy[:], xy[:], fxy[:])

                # ---- window starts s2[ax][slot] = clamp(floor-3, 0, 504) ----
                s2 = pool.tile([P, 2, SLOTS], f32)
                v.tensor_scalar(
                    _fap(s2, [[SLOTS, 2], [1, SLOTS]]),
                    _fap(fxy, [[1, 2], [2, SLOTS]]),
                    -3.0, 0.0, Alu.add, Alu.max)
                v.tensor_scalar(s2[:], s2[:], float(W - RB), None, Alu.min)

                # ---- gather element offsets: (sy*512 + sx)*24 + img_base ----
                # exact in f32: every value is a multiple of 24 below 2^25.
                u_t = pool.tile([P, SLOTS], f32)
                v.tensor_scalar(
                    u_t[:], _fap(s2, [[1, SLOTS]], SLOTS), float(W), None, Alu.mult)
                v.tensor_tensor(u_t[:], u_t[:], _fap(s2, [[1, SLOTS]]), op=Alu.add)
                idxf = pool.tile([P, SLOTS], f32)
                v.tensor_scalar(idxf[:], u_t[:], float(XS), None, Alu.mult)
                v.tensor_tensor(
                    idxf[:], idxf[:], _fap(m, [[1, SLOTS]], O_IB), op=Alu.add)
                idx = pool.tile([P, SLOTS], i32)
                v.tensor_copy(idx[:], idxf[:])

                # ---- indirect gather: one 192-elem contiguous run per sample
                # (HW SWDGE pairs ONE index per partition-row descriptor per
                # call, so one call per slot) ----
                patches = pool.tile([P, SLOTS, PATCH], f32)
                for s in range(SLOTS):
                    g.indirect_dma_start(
                        out=_fap(patches, [[1, PATCH]], s * PATCH),
                        out_offset=None,
                        in_=img[:],
                        in_offset=bass.IndirectOffsetOnAxis(
                            ap=_fap(idx, [[1, 1]], s), axis=0),
                    )

                # ---- blended 8-tap kernel kkb[ax][slot][u] = kk8 + w*dk8 ----
                kkb = pool.tile([P, 2, SLOTS, 8], f32)
                v.tensor_tensor(
                    _fap(kkb, [[SLOTS * 8, 2], [8, SLOTS], [1, 8]]),
                    _fap(wxy, [[1, 2], [2, SLOTS], [0, 8]]),
                    _fap(m, [[0, 2], [0, SLOTS], [1, 8]], O_DK8),
                    op=Alu.mult)
                v.tensor_tensor(
                    _fap(kkb, [[8, 2 * SLOTS], [1, 8]]),
                    _fap(kkb, [[8, 2 * SLOTS], [1, 8]]),
                    _fap(m, [[0, 2 * SLOTS], [1, 8]], O_KK8),
                    op=Alu.add)

                # ---- raw taps T[ax][slot][u] = floor + (u-3); reflect ----
                t_t = pool.tile([P, 2, SLOTS, 8], f32)
                v.tensor_tensor(
                    _fap(t_t, [[SLOTS * 8, 2], [8, SLOTS], [1, 8]]),
                    _fap(fxy, [[1, 2], [2, SLOTS], [0, 8]]),
                    _fap(m, [[0, 2], [0, SLOTS], [1, 8]], O_IM3),
                    op=Alu.add)
                neg = pool.tile([P, 2, SLOTS, 8], f32)
                v.tensor_scalar(neg[:], t_t[:], -1.0, None, Alu.mult)
                a_t = pool.tile([P, 2, SLOTS, 8], f32)
                v.tensor_tensor(a_t[:], t_t[:], neg[:], op=Alu.max)
                b_t = pool.tile([P, 2, SLOTS, 8], f32)
                v.tensor_scalar(
                    b_t[:], t_t[:], -1.0, float(2 * (W - 1)), Alu.mult, Alu.add)
                r_t = pool.tile([P, 2, SLOTS, 8], f32)
                v.tensor_tensor(r_t[:], a_t[:], b_t[:], op=Alu.min)

                # ---- window-relative tap Z = R - s2[ax][slot], in [0,8) ----
                z_t = pool.tile([P, 2, SLOTS, 8], f32)
                v.tensor_tensor(
                    _fap(z_t, [[SLOTS * 8, 2], [8, SLOTS], [1, 8]]),
                    _fap(r_t, [[SLOTS * 8, 2], [8, SLOTS], [1, 8]]),
                    _fap(s2, [[SLOTS, 2], [1, SLOTS], [0, 8]]),
                    op=Alu.subtract)

                # ---- scatter: Wv[g][v] = sum_u kkb[g][u] * (Z[g][u] == v) ----
                eq = pool.tile([P, 2 * SLOTS, 8, 8], f32)   # [g][v][u]
                v.tensor_tensor(
                    eq[:],
                    _fap(z_t, [[8, 2 * SLOTS], [0, 8], [1, 8]]),
                    _fap(m, [[0, 2 * SLOTS], [1, 8], [0, 8]], O_IOTA),
                    op=Alu.is_equal)
                v.tensor_tensor(
                    eq[:], eq[:],
                    _fap(kkb, [[8, 2 * SLOTS], [0, 8], [1, 8]]),
                    op=Alu.mult)
                wv = pool.tile([P, 2, SLOTS, 8], f32)       # [ax][slot][v]
                v.tensor_reduce(
                    out=_fap(wv, [[1, 2 * SLOTS * 8]]),
                    in_=eq[:], axis=Ax.X, op=Alu.add)

                # ---- outer product wq[slot][x][y] = Wx[x] * Wy[y] ----
                wq = pool.tile([P, SLOTS, 8, 8], f32)
                v.tensor_tensor(
                    wq[:],
                    _fap(wv, [[8, SLOTS], [1, 8], [0, 8]]),
                    _fap(wv, [[8, SLOTS], [0, 8], [1, 8]], SLOTS * 8),
                    op=Alu.mult)

                # ---- apply weights + MSE partial (Pool engine tail) ----
                # dummy copy absorbs the gather-DMA sem wait (one wait per
                # instruction after split_multi_waits).
                dummy = pool.tile([P, 1], f32)
                g.tensor_copy(dummy[:], _fap(patches, [[1, 1]]))
                tmp = pool.tile([P, SLOTS, 64, NCH], f32)
                g.tensor_tensor(
                    tmp[:],
                    _fap(patches, [[PATCH, SLOTS], [NCH, 64], [1, NCH]]),
                    _fap(wq, [[64, SLOTS], [1, 64], [0, NCH]]),
                    op=Alu.mult)
                tgt = pool.tile([P, SLOTS, NCH], f32)
                v.tensor_reduce(
                    out=_fap(tgt, [[1, SLOTS * NCH]]),
                    in_=_fap(tmp, [[PATCH, SLOTS], [1, NCH], [NCH, 64]]),
                    axis=Ax.X, op=Alu.add)
                diff = pool.tile([P, SLOTS, NCH], f32)
                v.tensor_tensor(
                    diff[:], tgt[:], _fap(m, [[NCH, SLOTS], [1, NCH]], O_COL),
                    op=Alu.subtract)
                sq = pool.tile([P, SLOTS, NCH], f32)
                v.tensor_tensor(sq[:], diff[:], diff[:], op=Alu.mult)
                part = pool.tile([P, 1], f32)
                v.tensor_reduce(
                    out=part[:], in_=_fap(sq, [[1, SLOTS * NCH]]),
                    axis=Ax.X, op=Alu.add)

                nc.sync.dma_start(out=partial[:], in_=part[:])

    split_multi_waits(nc)
    return nc


def make_banded(ref_imgs):
    """[B, 3, H, W] -> per-image banded layout D[b][band][x][row][ch]."""
    arr = np.ascontiguousarray(
        ref_imgs.astype(np.float32).transpose(0, 2, 3, 1))  # [B, y, x, c]
    sy, sx, sc = arr.strides[1:]
    bands = np.lib.stride_tricks.as_strided(
        arr, shape=(B, NB, RB, W, NCH),
        strides=(arr.strides[0], sy, sy, sx, sc))
    return np.ascontiguousarray(bands.transpose(0, 1, 3, 2, 4))  # [B,b,x,r,c]


def make_meta(pred_shard):
    """Per-core [P, META_W] meta tensor from the [BPC, L, 8] predictions
    shard.  Sample i = slot*P + p."""
    flat = np.ascontiguousarray(pred_shard.reshape(NS, 8).astype(np.float32))
    meta = np.zeros((P, META_W), dtype=np.float32)
    pos = flat[:, :2].reshape(SLOTS, P, 2).transpose(1, 0, 2)     # [P,SLOTS,2]
    col = flat[:, 5:8].reshape(SLOTS, P, 3).transpose(1, 0, 2)    # [P,SLOTS,3]
    meta[:, O_POS:O_POS + 4] = pos.reshape(P, 4)
    meta[:, O_COL:O_COL + 6] = col.reshape(P, 6)
    kk8 = np.zeros(8, dtype=np.float32)
    kk8[:KS] = _gauss_kernel_np()
    meta[:, O_KK8:O_KK8 + 8] = kk8[None, :]
    dk8 = np.concatenate(([0.0], kk8[:7])).astype(np.float32) - kk8
    meta[:, O_DK8:O_DK8 + 8] = dk8[None, :]
    meta[:, O_IM3:O_IM3 + 8] = (np.arange(8, dtype=np.float32) - 3.0)[None, :]
    meta[:, O_IOTA:O_IOTA + 8] = np.arange(8, dtype=np.float32)[None, :]
    p_idx = np.arange(P)
    for slot in range(SLOTS):
        img_i = (slot * P + p_idx) // L
        meta[:, O_IB + slot] = (img_i * IMS).astype(np.float32)
    return meta


def make_in_maps(predictions, ref_imgs):
    banded = make_banded(np.asarray(ref_imgs))
    in_maps = []
    for k in range(NCORES):
        img_shard = banded[k * BPC:(k + 1) * BPC].reshape(-1, 1)
        meta = make_meta(predictions[k * BPC:(k + 1) * BPC])
        in_maps.append({"img": img_shard, "meta": meta})
    return in_maps


_NC_CACHE = {}


def get_nc(repeat=1):
    key = ("nc", repeat)
    if key not in _NC_CACHE:
        _NC_CACHE[key] = build_bass(repeat=repeat)
    return _NC_CACHE[key]


def _reduce_results(res):
    total = np.float64(0.0)
    for r in res.results:
        total += np.float64(r["partial"].sum(dtype=np.float64))
    return np.float32(total / (B * L * NCH))


def kernel(predictions, ref_imgs):
    predictions = np.asarray(predictions)
    ref_imgs = np.asarray(ref_imgs)
    nc = get_nc()
    in_maps = make_in_maps(predictions, ref_imgs)
    res = run_bass_kernel_spmd(nc, in_maps, list(range(NCORES)))
    return _reduce_results(res)


def run_profiled(predictions, ref_imgs):
    """Like kernel(), but traces with neuron-profile; returns (loss, results)."""
    predictions = np.asarray(predictions)
    ref_imgs = np.asarray(ref_imgs)
    nc = get_nc()
    in_maps = make_in_maps(predictions, ref_imgs)
    res = run_bass_kernel_spmd(
        nc, in_maps, list(range(NCORES)), trace=True)
    return _reduce_results(res), res
